# revision 1
# baseline (speedup 1.0000x reference)
"""DigitCaps dynamic-routing kernel for 8 TRN2 NeuronCores.

Strategy: shard the C=1152 input capsules across the 8 cores (144 each) and
keep the full batch B=256 on every core.  The routing iterations use the
factored form (never materializing u_hat = x @ W, which would be 189 MB):

  s[b,u,o]    = sum_{c,i} x[b,i,c] * (coef[c,u] * W[c,u,o,i])     (matmul, K=(c,i))
  v           = squash(s)
  G[ci,uo]    = sum_b x[b,i,c] * v[b,u,o]                          (matmul, K=b)
  agr[c,u]    = (1/B) * sum_{o,i} W[c,u,o,i] * G[(c,i),(u,o)]      (mult + selector matmul)
  b_logits   += agr ; coef = softmax_u(b_logits)                   (tiny, c-local)

Only cross-core traffic: AllGather of the per-core partial s ([256,160] f32)
once per routing iteration (4 total).  The c-sharded agreement/logits state is
fully core-local.  Iteration 1's uniform coef=0.1 is folded into a 0.1
pre-scale of the x operand used by the s-matmul (and cancelled for later
iterations by scaling the coefficient-expansion constant by 10).

Precision: all s/G matmuls run as float32r (~2 cyc/row measured vs 4+ for
fp32; free dims padded to 256), including the final iteration's — measured
output error stays ~3e-4.  The squash, logits, softmax and coefficient
expansion stay fp32; the collective wire is fp16.
"""

import os
import sys

# Prefer the Mesh collective algorithm for the small (80KB) AllGathers: RDH
# measured ~12us vs Mesh ~8us at this size.  Harmless if the runtime ignores it.
os.environ.setdefault("NEURON_RT_DBG_RDH_CC", "0")

if "/opt/trn_rl_repo" not in sys.path:
    sys.path.insert(0, "/opt/trn_rl_repo")

import numpy as np

import concourse.bacc as bacc
import concourse.tile as tile
from concourse import mybir
from concourse.bass_utils import run_bass_kernel_spmd

F32 = mybir.dt.float32
F32R = mybir.dt.float32r
F16 = mybir.dt.float16
WIRE_DT = F16  # collective wire dtype (algorithm is RDH for either dtype; fp16 halves the gather)

B = 256          # batch
IU = 8           # in_unit (i)
C = 1152         # input capsules
U = 10           # output capsules
O = 16           # unit size
N_CORES = 8
CL = C // N_CORES          # 144 local capsules
CI = CL * IU               # 1152 local (c,i) rows
K = CI // 128              # 9 contraction tiles
UO = U * O                 # 160
UOP = 256                  # padded free dim so float32r runs at 1 cyc/row
NROUTE = 4
N_WARM = 24   # PE keep-warm dummy matmuls per routing iteration

# matmul dtype for the coefficient-path matmuls (s iters 0-2, G)
FAST_LAST_S = True  # fp32r on the final s-matmul too (~4us tail saving, ~+2e-4 err)


def _mm(nc, out, lhsT, rhs, start, stop, fast):
    if fast:
        lhsT = lhsT.bitcast(F32R)
        rhs = rhs.bitcast(F32R)
    nc.tensor.matmul(out, lhsT=lhsT, rhs=rhs, start=start, stop=stop)


def _build_program():
    nc = bacc.Bacc(
        "TRN2",
        target_bir_lowering=False,
        debug=False,
        enable_asserts=False,
        num_devices=N_CORES,
    )

    xp_d = nc.dram_tensor("xp", [128, K * B], F32, kind="ExternalInput").ap()
    xb_d = nc.dram_tensor("xb", [128, 2 * CI], F32, kind="ExternalInput").ap()
    w1_d = nc.dram_tensor("w1", [128, K * UOP], F32, kind="ExternalInput").ap()
    zc_d = nc.dram_tensor("zc", [128, K * (UOP - UO)], F32, kind="ExternalInput").ap()
    sel_d = nc.dram_tensor("sel", [128, 16], F32, kind="ExternalInput").ap()
    exp_d = nc.dram_tensor("exp16", [16, 128], F32, kind="ExternalInput").ap()
    out_d = nc.dram_tensor("out", [B, U, O, 1], F32, kind="ExternalOutput").ap()

    with tile.TileContext(nc) as tc:
        with (
            tc.tile_pool(name="persist", bufs=1) as pp,
            tc.tile_pool(name="work", bufs=2) as wp,
            tc.tile_pool(name="sps", bufs=1, space="PSUM") as sps,
            tc.tile_pool(name="gps", bufs=1, space="PSUM") as gps,
            tc.tile_pool(name="aps", bufs=1, space="PSUM") as aps,
            tc.tile_pool(name="cxps", bufs=1, space="PSUM") as cxps,
            tc.tile_pool(name="dram", bufs=2, space="DRAM") as dram,
        ):
            # ---- persistent tiles; padded regions zeroed once ----
            xp_s = pp.tile([128, K * B], F32, tag="xp")
            xb_s = pp.tile([128, 2 * CI], F32, tag="xb")
            w1_s = pp.tile([128, K * UOP], F32, tag="w1")     # [.., k*256+uo], pad 160:256
            weff_s = pp.tile([128, K * UOP], F32, tag="weff")
            v_s = pp.tile([128, 2 * UOP], F32, tag="v")       # [.., t*256+uo]
            v_last = pp.tile([128, 2 * UO], F32, tag="vlast")
            weff_last = pp.tile([128, K * UO], F32, tag="wefflast")
            pm2_s = pp.tile([128, K * U], F32, tag="pm2")     # o-reduced G*W
            sel_s = pp.tile([128, 16], F32, tag="sel")
            exp_s = pp.tile([16, 128], F32, tag="exp16")
            b_state = pp.tile([16, K * U], F32, tag="bstate")
            scr = pp.tile([128, 2], F32, tag="scr")           # ACT table prewarm scratch

            nc.gpsimd.memset(scr[:], 1.0)


            for j in range(3):
                kb3, ku3 = 3 * B, 3 * UOP
                nc.sync.dma_start(
                    xp_s[:, j * kb3:(j + 1) * kb3].bitcast(F32R),
                    xp_d[:, j * kb3:(j + 1) * kb3].bitcast(F32R),
                )
                nc.scalar.dma_start(
                    w1_s[:, j * ku3:(j + 1) * ku3].bitcast(F32R),
                    w1_d[:, j * ku3:(j + 1) * ku3].bitcast(F32R),
                )
            nc.scalar.dma_start(
                weff_s[:].rearrange("p (k q) -> p k q", k=K)[:, :, UO:].bitcast(F32R),
                zc_d.rearrange("p (k q) -> p k q", k=K).bitcast(F32R),
            )
            nc.scalar.dma_start(
                v_s[:].rearrange("p (t q) -> p t q", t=2)[:, :, UO:].bitcast(F32R),
                zc_d[:, : 2 * (UOP - UO)].rearrange("p (t q) -> p t q", t=2).bitcast(F32R),
            )
            nc.sync.dma_start(xb_s[:, :CI].bitcast(F32R), xb_d[:, :CI].bitcast(F32R))
            nc.sync.dma_start(xb_s[:, CI:].bitcast(F32R), xb_d[:, CI:].bitcast(F32R))
            nc.scalar.dma_start(sel_s[:], sel_d)
            nc.scalar.dma_start(exp_s[:], exp_d)

            weff = w1_s  # iteration 0 uses raw W (coef folded into xp scale)

            for r in range(NROUTE):
                last = r == NROUTE - 1
                fast_s = not last or FAST_LAST_S
                # ---- s partial: [b,(u,o)] += xp^T @ weff over (c,i) tiles ----
                wdt = WIRE_DT
                s_stage = wp.tile([128, 2 * UO], wdt, tag="s_stage" + ("_l" if last else ""))
                wstride = UOP if fast_s else UO
                s_ps = sps.tile([128, 2 * UOP], F32, tag="s_ps")
                for g in range(2):
                    for k in range(K):
                        out_ap = s_ps[:, g * UOP:(g + 1) * UOP]
                        _mm(
                            nc, out_ap if fast_s else out_ap[:, :UO],
                            xp_s[:, k * B + g * 128: k * B + (g + 1) * 128],
                            weff[:, k * wstride: k * wstride + wstride],
                            start=(k == 0), stop=(k == K - 1), fast=fast_s,
                        )
                    nc.vector.tensor_copy(
                        s_stage[:, g * UO:(g + 1) * UO],
                        s_ps[:, g * UOP: g * UOP + UO],
                    )

                # ---- AllGather partials, local 8-way tree sum ----
                cc_in = dram.tile([B, UO], wdt, tag="cc_in" + ("_l" if last else ""))
                cc_out = dram.tile(
                    [N_CORES * B, UO], wdt, tag="cc_out" + ("_l" if last else ""),
                    addr_space="Shared",
                )
                nc.sync.dma_start(
                    cc_in.opt().rearrange("(g p) f -> p g f", g=2), s_stage[:]
                )
                nc.gpsimd.collective_compute(
                    "AllGather",
                    mybir.AluOpType.bypass,
                    replica_groups=[list(range(N_CORES))],
                    ins=[cc_in.opt()],
                    outs=[cc_out.opt()],
                )
                sg_s = wp.tile([128, N_CORES * 2 * UO], wdt, tag="sg" + ("_l" if last else ""))
                cc_view = cc_out.opt().rearrange(
                    "(q g p) f -> q p g f", q=8, g=2
                )  # q: rank, g: b-half
                qf = 2 * UO
                engs = [nc.sync, nc.scalar, nc.gpsimd, nc.sync,
                        nc.scalar, nc.gpsimd, nc.sync, nc.scalar]
                for q in range(8):
                    engs[q].dma_start(sg_s[:, q * qf:(q + 1) * qf], cc_view[q])
                t1 = wp.tile([128, 4 * 2 * UO], F32, tag="t1")
                nc.vector.tensor_add(t1[:], sg_s[:, :4 * 2 * UO], sg_s[:, 4 * 2 * UO:])
                t2 = wp.tile([128, 2 * 2 * UO], F32, tag="t2")
                nc.vector.tensor_add(t2[:], t1[:, :2 * 2 * UO], t1[:, 2 * 2 * UO:])
                s_sb = wp.tile([128, 2 * UO], F32, tag="s_sb")
                nc.vector.tensor_add(s_sb[:], t2[:, :2 * UO], t2[:, 2 * UO:])

                # ---- squash: v = s * sqrt(n2) / (1 + n2) ----
                sq = wp.tile([128, 2 * UO], F32, tag="sq")
                nc.vector.tensor_mul(sq[:], s_sb[:], s_sb[:])
                n2 = wp.tile([128, 2 * U], F32, tag="n2")
                nc.vector.reduce_sum(
                    n2[:], sq[:].rearrange("p (t u o) -> p (t u) o", t=2, u=U),
                    axis=mybir.AxisListType.X,
                )
                rt = wp.tile([128, 2 * U], F32, tag="rt")
                nc.scalar.sqrt(rt[:], n2[:])
                if not last:
                    # prewarm the Exp ACT table while G/sel run (dep on rt orders it)
                    nc.scalar.activation(
                        scr[:, 1:2], rt[:, 0:1], mybir.ActivationFunctionType.Exp
                    )
                dn = wp.tile([128, 2 * U], F32, tag="dn")
                nc.vector.tensor_scalar_add(dn[:], n2[:], 1.0)
                rd = wp.tile([128, 2 * U], F32, tag="rd")
                nc.vector.reciprocal(rd[:], dn[:])
                f = wp.tile([128, 2 * U], F32, tag="f")
                nc.vector.tensor_mul(f[:], rt[:], rd[:])
                if last:
                    v_out = v_last[:].rearrange("p (t u o) -> p t u o", t=2, u=U)
                else:
                    v_out = v_s[:].rearrange("p (t q o) -> p t q o", t=2, q=16)[:, :, :U, :].bitcast(F32R)
                nc.vector.tensor_mul(
                    v_out,
                    s_sb[:].rearrange("p (t u o) -> p t u o", t=2, u=U),
                    f[:].rearrange("p (t u) -> p t u", t=2).unsqueeze(3).broadcast_to((128, 2, U, O)),
                )

                if last:
                    nc.sync.dma_start(
                        out_d.rearrange("(g p) u o one -> p g (u o one)", g=2),
                        v_last[:].rearrange("p (t f) -> p t f", t=2),
                    )
                    break

                # ---- G[(c,i),(u,o)] = sum_b x*v ; pm2 = sum_o G*W ; agr = sel^T pm2 ----
                g_ps = gps.tile([128, K * UOP], F32, tag="g_ps")
                for m in range(K):
                    for t in range(2):
                        _mm(
                            nc, g_ps[:, m * UOP:(m + 1) * UOP],
                            xb_s[:, t * CI + m * 128: t * CI + (m + 1) * 128],
                            v_s[:, t * UOP:(t + 1) * UOP],
                            start=(t == 0), stop=(t == 1), fast=True,
                        )
                pm = wp.tile([128, K * UO], F32, tag="pm")
                for j in range(3):
                    nc.vector.tensor_mul(
                        pm[:].rearrange("p (k f) -> p k f", k=K)[:, 3 * j:3 * j + 3, :],
                        g_ps[:].rearrange("p (k q) -> p k q", k=K)[:, 3 * j:3 * j + 3, :UO],
                        w1_s[:].rearrange("p (k q) -> p k q", k=K)[:, 3 * j:3 * j + 3, :UO],
                    )
                    nc.vector.reduce_sum(
                        pm2_s[:].rearrange("p (k u) -> p k u", k=K)[:, 3 * j:3 * j + 3, :],
                        pm[:].rearrange("p (k u o) -> p k u o", k=K, u=U)[:, 3 * j:3 * j + 3, :, :],
                        axis=mybir.AxisListType.X,
                    )
                a_ps = aps.tile([16, K * U], F32, tag="a_ps")
                nc.tensor.matmul(a_ps[:], lhsT=sel_s[:], rhs=pm2_s[:], start=True, stop=True)

                # ---- logits update + softmax over u (c-local, tiny) ----
                if r == 0:
                    nc.vector.tensor_copy(b_state[:], a_ps[:])
                else:
                    nc.vector.tensor_add(b_state[:], b_state[:], a_ps[:])
                eb = wp.tile([16, K * U], F32, tag="eb")
                nc.scalar.activation(eb[:], b_state[:], mybir.ActivationFunctionType.Exp)
                # prewarm the Sqrt ACT table for the next squash
                nc.scalar.activation(
                    scr[:16, 0:1], eb[:, 0:1], mybir.ActivationFunctionType.Sqrt
                )
                den = wp.tile([16, K], F32, tag="den")
                nc.vector.reduce_sum(
                    den[:], eb[:].rearrange("p (k u) -> p k u", k=K),
                    axis=mybir.AxisListType.X,
                )
                rden = wp.tile([16, K], F32, tag="rden")
                nc.vector.reciprocal(rden[:], den[:])
                cnorm = wp.tile([16, K * U], F32, tag="cnorm")
                nc.vector.tensor_mul(
                    cnorm[:].rearrange("p (k u) -> p k u", k=K),
                    eb[:].rearrange("p (k u) -> p k u", k=K),
                    rden[:].unsqueeze(2).broadcast_to((16, K, U)),
                )

                # ---- expand coef to (c,i) partitions; W_eff = W * coef ----
                cx_ps = cxps.tile([128, K * U], F32, tag="cx")
                nc.tensor.matmul(cx_ps[:], lhsT=exp_s[:], rhs=cnorm[:], start=True, stop=True)
                cx_sb = wp.tile([128, K * U], F32, tag="cx_sb")
                nc.vector.tensor_copy(cx_sb[:], cx_ps[:])
                if r < NROUTE - 2 or FAST_LAST_S:
                    weff_out = weff_s[:].rearrange("p (k q o) -> p k q o", k=K, q=16)[:, :, :U, :].bitcast(F32R)
                    weff = weff_s
                else:
                    weff_out = weff_last[:].rearrange("p (k u o) -> p k u o", k=K, u=U)
                    weff = weff_last
                nc.vector.tensor_mul(
                    weff_out,
                    w1_s[:].rearrange("p (k q o) -> p k q o", k=K, q=16)[:, :, :U, :],
                    cx_ps[:].rearrange("p (k u) -> p k u", k=K).unsqueeze(3).broadcast_to((128, K, U, O)),
                )

    nc.compile()
    return nc


_PROGRAM_CACHE = {}


def _get_program():
    if "nc" not in _PROGRAM_CACHE:
        _PROGRAM_CACHE["nc"] = _build_program()
    return _PROGRAM_CACHE["nc"]


def _make_in_maps(x, W):
    x = np.ascontiguousarray(x, dtype=np.float32)
    W = np.ascontiguousarray(W, dtype=np.float32)
    sel = np.zeros((128, 16), dtype=np.float32)
    for p in range(128):
        sel[p, p // IU] = 1.0 / B
    exp16 = np.zeros((16, 128), dtype=np.float32)
    for p in range(128):
        exp16[p // IU, p] = 10.0  # cancels the 0.1 pre-scale of xp

    in_maps = []
    for core in range(N_CORES):
        c0 = core * CL
        xc = x[:, :, c0:c0 + CL]                    # [B, I, CL]
        Wc = W[c0:c0 + CL]                          # [CL, U, O, I]
        # xp[p, k*B + b] = 0.1 * x[b, i, c], ci = k*128+p = c_rel*8+i
        xp = 0.1 * xc.transpose(2, 1, 0).reshape(CI, B)
        xp = np.ascontiguousarray(
            xp.reshape(K, 128, B).transpose(1, 0, 2).reshape(128, K * B)
        )
        # xb[p, t*CI + ci] = x[t*128+p, i, c]
        xb = xc.transpose(0, 2, 1).reshape(B, CI)
        xb = np.ascontiguousarray(
            xb.reshape(2, 128, CI).transpose(1, 0, 2).reshape(128, 2 * CI)
        )
        # w1[p, k*UOP + uo] = W[c, u, o, i], zero-padded to UOP per k-tile
        w1 = Wc.transpose(0, 3, 1, 2).reshape(CI, UO).reshape(K, 128, UO)
        w1p = np.zeros((128, K, UOP), dtype=np.float32)
        w1p[:, :, :UO] = w1.transpose(1, 0, 2)
        w1p = np.ascontiguousarray(w1p.reshape(128, K * UOP))
        zc = np.zeros((128, K * (UOP - UO)), dtype=np.float32)
        in_maps.append(
            {"xp": xp, "xb": xb, "w1": w1p, "sel": sel, "exp16": exp16, "zc": zc}
        )
    return in_maps


def kernel(x, W, _trace=False, _trace_kwargs=None):
    nc = _get_program()
    in_maps = _make_in_maps(x, W)
    res = run_bass_kernel_spmd(
        nc, in_maps, core_ids=list(range(N_CORES)), trace=_trace,
        **(_trace_kwargs or {}),
    )
    out = res.results[0]["out"].astype(np.float32).reshape(B, U, O, 1)
    if _trace:
        kernel.last_results = res
    return out



# revision 12
# speedup vs baseline: 1.1883x; 1.1883x over previous
"""DigitCaps dynamic-routing kernel for 8 TRN2 NeuronCores.

Strategy: shard the C=1152 input capsules across the 8 cores (144 each) and
keep the full batch B=256 on every core.  The routing iterations use the
factored form (never materializing u_hat = x @ W, which would be 189 MB):

  s[b,u,o]    = sum_{c,i} x[b,i,c] * (coef[c,u] * W[c,u,o,i])     (matmul, K=(c,i))
  v           = squash(s)
  G[ci,uo]    = sum_b x[b,i,c] * v[b,u,o]                          (matmul, K=b)
  agr[c,u]    = (1/B) * sum_{o,i} W[c,u,o,i] * G[(c,i),(u,o)]      (mult + selector matmul)
  b_logits   += agr ; coef = softmax_u(b_logits)                   (tiny, c-local)

Only cross-core traffic: AllGather of the per-core partial s ([256,160] fp16)
once per routing iteration (4 total).  The c-sharded agreement/logits state is
fully core-local.  Iteration 1's uniform coef=0.1 is folded into a 0.1
pre-scale of the x operand used by the s-matmul (and cancelled for later
iterations by scaling the coefficient-expansion constant by 10).

v2 changes vs the fp32r baseline:
- all matmuls run in bf16 (1 cyc/row, 1-pass LDWEIGHTS, no 256-wide free-dim
  padding needed); inputs are shipped to HBM as bf16, halving the upload.
- the post-AllGather gather of the 8 rank partials is ONE strided DMA into
  [128, 8*2*160] instead of 8 small DMAs on 3 queues; the 8-way tree-sum
  runs in fp16 (2x DVE mode for the first two levels).
- the agreement -> softmax -> coef-expansion -> W_eff -> next-iteration
  s-matmul chain is pipelined in 3 chunks of 3 k-tiles, with the elementwise
  muls alternating between the Vector and GpSimd engines so DVE reductions
  overlap GpSimd muls and TensorE matmuls.
- dummy fp16 matmuls (dependent on the gathered tile, so they fire right
  after each AllGather lands) keep the PE HAM clock-gate warm so the real
  G/s matmuls run at 2.4 GHz instead of 1.2 GHz.
"""

import os
import sys

# Prefer the Mesh collective algorithm for the small (80KB) AllGathers if the
# runtime sees this env (no-op under the axon-tunneled remote runtime).
os.environ.setdefault("NEURON_RT_DBG_RDH_CC", "0")

if "/opt/trn_rl_repo" not in sys.path:
    sys.path.insert(0, "/opt/trn_rl_repo")

import numpy as np

import concourse.bacc as bacc
import concourse.tile as tile
from concourse import mybir
from concourse.bass_utils import run_bass_kernel_spmd

F32 = mybir.dt.float32
F16 = mybir.dt.float16
BF16 = mybir.dt.bfloat16
WIRE_DT = F16  # collective wire dtype

B = 256          # batch
IU = 8           # in_unit (i)
C = 1152         # input capsules
U = 10           # output capsules
O = 16           # unit size
N_CORES = 8
CL = C // N_CORES          # 144 local capsules
CI = CL * IU               # 1152 local (c,i) rows
K = CI // 128              # 9 contraction tiles
UO = U * O                 # 160
NROUTE = 4
NCH = 3                    # k-tiles per agreement chunk (K/3)
N_WARM = 28                # PE keep-warm dummy matmuls per routing iteration
GATHER_ONE_DMA = True
DEBUG_TAPS = False


def _build_program():
    nc = bacc.Bacc(
        "TRN2",
        target_bir_lowering=False,
        debug=False,
        enable_asserts=False,
        num_devices=N_CORES,
    )

    if DEBUG_TAPS:
        dbg_s16 = nc.dram_tensor("dbg_s16", [128, 2 * UO], F16, kind="ExternalOutput").ap()
        dbg_sg = nc.dram_tensor("dbg_sg", [128, 16 * UO], F16, kind="ExternalOutput").ap()
        dbg_ssb = nc.dram_tensor("dbg_ssb", [128, 2 * UO], F16, kind="ExternalOutput").ap()
    xp_d = nc.dram_tensor("xp", [128, K * B], BF16, kind="ExternalInput").ap()
    xb_d = nc.dram_tensor("xb", [128, 2 * CI], BF16, kind="ExternalInput").ap()
    w1_d = nc.dram_tensor("w1", [128, K * UO], BF16, kind="ExternalInput").ap()
    sel_d = nc.dram_tensor("sel", [128, 16], BF16, kind="ExternalInput").ap()
    exp_d = nc.dram_tensor("exp16", [16, 128], BF16, kind="ExternalInput").ap()
    out_d = nc.dram_tensor("out", [B, U, O, 1], F32, kind="ExternalOutput").ap()

    with tile.TileContext(nc) as tc:
        with (
            tc.tile_pool(name="persist", bufs=1) as pp,
            tc.tile_pool(name="work", bufs=2) as wp,
            tc.tile_pool(name="sps", bufs=1, space="PSUM") as sps,
            tc.tile_pool(name="gps", bufs=1, space="PSUM") as gps,
            tc.tile_pool(name="aps", bufs=1, space="PSUM") as aps,
            tc.tile_pool(name="cxps", bufs=1, space="PSUM") as cxps,
            tc.tile_pool(name="dram", bufs=2, space="DRAM") as dram,
        ):
            # ---- persistent tiles ----
            xp_s = pp.tile([128, K * B], BF16, tag="xp")
            xb_s = pp.tile([128, 2 * CI], BF16, tag="xb")
            w1_s = pp.tile([128, K * UO], BF16, tag="w1")
            weff_s = pp.tile([128, K * UO], BF16, tag="weff")
            v_s = pp.tile([128, 2 * UO], BF16, tag="v")
            v_last = pp.tile([128, 2 * UO], F32, tag="vlast")
            pm2_s = pp.tile([128, K * U], BF16, tag="pm2")
            sel_s = pp.tile([128, 16], BF16, tag="sel")
            exp_s = pp.tile([16, 128], BF16, tag="exp16")
            b_state = pp.tile([16, K * U], F32, tag="bstate")
            scr = pp.tile([128, 2], F32, tag="scr")   # ACT table prewarm scratch

            nc.gpsimd.memset(scr[:], 1.0)

            # ---- input loads (chunked so the round-0 s-matmul starts early) ----
            for j in range(3):
                kb3, ku3 = 3 * B, 3 * UO
                nc.sync.dma_start(
                    xp_s[:, j * kb3:(j + 1) * kb3], xp_d[:, j * kb3:(j + 1) * kb3]
                )
                nc.scalar.dma_start(
                    w1_s[:, j * ku3:(j + 1) * ku3], w1_d[:, j * ku3:(j + 1) * ku3]
                )
            nc.scalar.dma_start(sel_s[:], sel_d)
            nc.scalar.dma_start(exp_s[:], exp_d)
            nc.scalar.dma_start(xb_s[:, :CI], xb_d[:, :CI])
            nc.scalar.dma_start(xb_s[:, CI:], xb_d[:, CI:])

            def s_mm(s_ps, weff, ks):
                # the two b-half accumulation groups live in separate 2KB PSUM
                # zero regions (offsets 0 and 512 f32): start=True marks the
                # whole zero region pending-zero, so interleaved groups must
                # not share a bank.
                for g in range(2):
                    for k in ks:
                        nc.tensor.matmul(
                            s_ps[:, g * 512: g * 512 + UO],
                            lhsT=xp_s[:, k * B + g * 128: k * B + (g + 1) * 128],
                            rhs=weff[:, k * UO:(k + 1) * UO],
                            start=(k == 0), stop=(k == K - 1),
                        )

            def stage_and_ag(s_ps):
                s16 = wp.tile([128, 2 * UO], WIRE_DT, tag="s16")
                nc.vector.tensor_copy(
                    s16[:].rearrange("p (g f) -> p g f", g=2),
                    s_ps[:].rearrange("p (g w) -> p g w", g=2)[:, :, :UO],
                )
                cc_in = dram.tile([B, UO], WIRE_DT, tag="cc_in")
                cc_out = dram.tile(
                    [N_CORES * B, UO], WIRE_DT, tag="cc_out", addr_space="Shared"
                )
                nc.sync.dma_start(
                    cc_in.opt().rearrange("(g p) f -> p g f", g=2), s16[:]
                )
                if DEBUG_TAPS:
                    nc.scalar.dma_start(dbg_s16, s16[:])
                nc.gpsimd.collective_compute(
                    "AllGather",
                    mybir.AluOpType.bypass,
                    replica_groups=[list(range(N_CORES))],
                    ins=[cc_in.opt()],
                    outs=[cc_out.opt()],
                )
                return cc_out

            # ---- round 0: s = (0.1 x)^T W, AllGather ----
            s_ps = sps.tile([128, 2 * 512], F32, tag="s_ps")
            s_mm(s_ps, w1_s, range(K))
            cc_out = stage_and_ag(s_ps)

            for rnd in range(1, NROUTE + 1):
                last = rnd == NROUTE

                # ---- gather the 8 rank partials (one strided DMA), tree-sum ----
                sg = wp.tile([128, 16 * UO], WIRE_DT, tag="sg")
                if GATHER_ONE_DMA:
                    nc.sync.dma_start(
                        sg[:].rearrange("p (q g f) -> p q g f", q=8, g=2),
                        cc_out.opt().rearrange("(q g p) f -> p q g f", q=8, g=2),
                    )
                else:
                    cc_view = cc_out.opt().rearrange("(q g p) f -> q p g f", q=8, g=2)
                    qf = 2 * UO
                    engs = [nc.sync, nc.scalar, nc.gpsimd, nc.sync,
                            nc.scalar, nc.gpsimd, nc.sync, nc.scalar]
                    for q in range(8):
                        engs[q].dma_start(sg[:, q * qf:(q + 1) * qf], cc_view[q])
                if not last:
                    # PE keep-warm: dummy fp16 matmuls gated on the gathered tile
                    # so they run during the tree-sum/squash window and ramp the
                    # HAM clock before the G/s matmul burst.  They scribble on
                    # s_ps, which the next round's s-matmul overwrites
                    # (start=True) anyway.
                    for wi in range(N_WARM):
                        nc.tensor.matmul(
                            s_ps[:, :UO],
                            lhsT=sg[:, :128],
                            rhs=sg[:, :UO],
                            start=True, stop=True,
                        )
                t1 = wp.tile([128, 8 * UO], WIRE_DT, tag="t1")
                nc.vector.tensor_add(t1[:], sg[:, :8 * UO], sg[:, 8 * UO:])
                t2 = wp.tile([128, 4 * UO], WIRE_DT, tag="t2")
                nc.vector.tensor_add(t2[:], t1[:, :4 * UO], t1[:, 4 * UO:])
                s_sb = wp.tile([128, 2 * UO], WIRE_DT, tag="s_sb")
                nc.vector.tensor_add(s_sb[:], t2[:, :2 * UO], t2[:, 2 * UO:])
                if DEBUG_TAPS and rnd == 1:
                    nc.scalar.dma_start(dbg_sg, sg[:])
                    nc.scalar.dma_start(dbg_ssb, s_sb[:])

                # ---- squash: v = s * sqrt(n2) / (1 + n2) ----
                sq = wp.tile([128, 2 * UO], F32, tag="sq")
                nc.vector.tensor_mul(sq[:], s_sb[:], s_sb[:])
                n2 = wp.tile([128, 2 * U], F32, tag="n2")
                nc.vector.reduce_sum(
                    n2[:], sq[:].rearrange("p (t u o) -> p (t u) o", t=2, u=U),
                    axis=mybir.AxisListType.X,
                )
                rt = wp.tile([128, 2 * U], F32, tag="rt")
                nc.scalar.sqrt(rt[:], n2[:])
                if not last:
                    # prewarm the Exp ACT table while G runs (dep on rt orders it)
                    nc.scalar.activation(
                        scr[:, 1:2], rt[:, 0:1], mybir.ActivationFunctionType.Exp
                    )
                dn = wp.tile([128, 2 * U], F32, tag="dn")
                nc.gpsimd.tensor_scalar_add(dn[:], n2[:], 1.0)
                rd = wp.tile([128, 2 * U], F32, tag="rd")
                nc.vector.reciprocal(rd[:], dn[:])
                f = wp.tile([128, 2 * U], F32, tag="f")
                nc.vector.tensor_mul(f[:], rt[:], rd[:])
                v_dst = v_last if last else v_s
                nc.vector.tensor_mul(
                    v_dst[:].rearrange("p (t u o) -> p t u o", t=2, u=U),
                    s_sb[:].rearrange("p (t u o) -> p t u o", t=2, u=U),
                    f[:].rearrange("p (t u) -> p t u", t=2).unsqueeze(3).broadcast_to((128, 2, U, O)),
                )

                if last:
                    nc.sync.dma_start(
                        out_d.rearrange("(g p) u o one -> p g (u o one)", g=2),
                        v_last[:].rearrange("p (t f) -> p t f", t=2),
                    )
                    break

                # ---- G[(c,i),(u,o)] = sum_b x*v, in 3 chunks of 3 k-tiles ----
                g_chunks = []
                for jc in range(3):
                    g_ps = gps.tile([128, NCH * UO], F32, tag=f"g_ps{jc}")
                    for mm in range(NCH):
                        m = NCH * jc + mm
                        for t in range(2):
                            nc.tensor.matmul(
                                g_ps[:, mm * UO:(mm + 1) * UO],
                                lhsT=xb_s[:, t * CI + m * 128: t * CI + (m + 1) * 128],
                                rhs=v_s[:, t * UO:(t + 1) * UO],
                                start=(t == 0), stop=(t == 1),
                            )
                    g_chunks.append(g_ps)

                # next round's s accumulator
                s_ps = sps.tile([128, 2 * 512], F32, tag="s_ps")

                # ---- per-chunk: agreement -> logits -> softmax -> W_eff -> s-matmul ----
                for j in range(3):
                    su = slice(j * NCH * U, (j + 1) * NCH * U)      # [3U] logits cols
                    suo = slice(j * NCH * UO, (j + 1) * NCH * UO)   # [3UO] weight cols

                    pm = wp.tile([128, NCH * UO], F32, tag=f"pm{j}")
                    nc.vector.tensor_mul(pm[:], g_chunks[j][:], w1_s[:, suo])
                    with nc.allow_low_precision(reason="routing logits tolerate bf16"):
                        nc.vector.reduce_sum(
                            pm2_s[:, su].rearrange("p (m u) -> p m u", m=NCH),
                            pm[:].rearrange("p (m u o) -> p m u o", m=NCH, u=U),
                            axis=mybir.AxisListType.X,
                        )
                    a_ps = aps.tile([16, NCH * U], F32, tag="a_ps")
                    nc.tensor.matmul(
                        a_ps[:], lhsT=sel_s[:], rhs=pm2_s[:, su], start=True, stop=True
                    )
                    if rnd == 1:
                        nc.vector.tensor_copy(b_state[:, su], a_ps[:])
                    else:
                        nc.vector.tensor_add(b_state[:, su], b_state[:, su], a_ps[:])
                    eb = wp.tile([16, NCH * U], F32, tag=f"eb{j}")
                    nc.scalar.activation(
                        eb[:], b_state[:, su], mybir.ActivationFunctionType.Exp
                    )
                    if j == 2:
                        # prewarm the Sqrt ACT table for the next squash
                        nc.scalar.activation(
                            scr[:16, 0:1], eb[:, 0:1], mybir.ActivationFunctionType.Sqrt
                        )
                    den = wp.tile([16, NCH], F32, tag=f"den{j}")
                    nc.vector.reduce_sum(
                        den[:], eb[:].rearrange("p (m u) -> p m u", m=NCH),
                        axis=mybir.AxisListType.X,
                    )
                    rden = wp.tile([16, NCH], F32, tag=f"rden{j}")
                    nc.vector.reciprocal(rden[:], den[:])
                    cn = wp.tile([16, NCH * U], BF16, tag=f"cn{j}")
                    nc.gpsimd.tensor_mul(
                        cn[:].rearrange("p (m u) -> p m u", m=NCH),
                        eb[:].rearrange("p (m u) -> p m u", m=NCH),
                        rden[:].unsqueeze(2).broadcast_to((16, NCH, U)),
                    )
                    cx_ps = cxps.tile([128, NCH * U], F32, tag="cx")
                    nc.tensor.matmul(
                        cx_ps[:], lhsT=exp_s[:], rhs=cn[:], start=True, stop=True
                    )
                    # GpSimd cannot read PSUM: stage cx through SBUF on ACT
                    cx_sb = wp.tile([128, NCH * U], F32, tag=f"cx_sb{j}")
                    nc.scalar.copy(cx_sb[:], cx_ps[:])
                    nc.gpsimd.tensor_mul(
                        weff_s[:, suo].rearrange("p (m u o) -> p m u o", m=NCH, u=U),
                        w1_s[:, suo].rearrange("p (m u o) -> p m u o", m=NCH, u=U),
                        cx_sb[:].rearrange("p (m u) -> p m u", m=NCH).unsqueeze(3).broadcast_to((128, NCH, U, O)),
                    )
                    s_mm(s_ps, weff_s, range(j * NCH, (j + 1) * NCH))

                cc_out = stage_and_ag(s_ps)

    nc.compile()
    return nc


_PROGRAM_CACHE = {}


def _get_program():
    if "nc" not in _PROGRAM_CACHE:
        _PROGRAM_CACHE["nc"] = _build_program()
    return _PROGRAM_CACHE["nc"]


def _make_in_maps(x, W):
    BF16_NP = mybir.dt.np(BF16)
    x = np.ascontiguousarray(x, dtype=np.float32)
    W = np.ascontiguousarray(W, dtype=np.float32)
    sel = np.zeros((128, 16), dtype=np.float32)
    for p in range(128):
        sel[p, p // IU] = 1.0 / B
    exp16 = np.zeros((16, 128), dtype=np.float32)
    for p in range(128):
        exp16[p // IU, p] = 10.0  # cancels the 0.1 pre-scale of xp
    sel = sel.astype(BF16_NP)
    exp16 = exp16.astype(BF16_NP)

    in_maps = []
    for core in range(N_CORES):
        c0 = core * CL
        xc = x[:, :, c0:c0 + CL]                    # [B, I, CL]
        Wc = W[c0:c0 + CL]                          # [CL, U, O, I]
        # xp[p, k*B + b] = 0.1 * x[b, i, c], ci = k*128+p = c_rel*8+i
        xp = 0.1 * xc.transpose(2, 1, 0).reshape(CI, B)
        xp = np.ascontiguousarray(
            xp.reshape(K, 128, B).transpose(1, 0, 2).reshape(128, K * B)
        ).astype(BF16_NP)
        # xb[p, t*CI + ci] = x[t*128+p, i, c]
        xb = xc.transpose(0, 2, 1).reshape(B, CI)
        xb = np.ascontiguousarray(
            xb.reshape(2, 128, CI).transpose(1, 0, 2).reshape(128, 2 * CI)
        ).astype(BF16_NP)
        # w1[p, k*UO + uo] = W[c, u, o, i]
        w1 = Wc.transpose(0, 3, 1, 2).reshape(CI, UO).reshape(K, 128, UO)
        w1 = np.ascontiguousarray(
            w1.transpose(1, 0, 2).reshape(128, K * UO)
        ).astype(BF16_NP)
        in_maps.append(
            {"xp": xp, "xb": xb, "w1": w1, "sel": sel, "exp16": exp16}
        )
    return in_maps


def kernel(x, W, _trace=False, _trace_kwargs=None):
    nc = _get_program()
    in_maps = _make_in_maps(x, W)
    res = run_bass_kernel_spmd(
        nc, in_maps, core_ids=list(range(N_CORES)), trace=_trace,
        **(_trace_kwargs or {}),
    )
    out = res.results[0]["out"].astype(np.float32).reshape(B, U, O, 1)
    if _trace:
        kernel.last_results = res
    return out


# revision 15
# speedup vs baseline: 1.3895x; 1.1692x over previous
"""DigitCaps dynamic-routing kernel for 8 TRN2 NeuronCores.

Strategy: shard the C=1152 input capsules across the 8 cores (144 each) and
keep the full batch B=256 on every core.  The routing iterations use the
factored form (never materializing u_hat = x @ W, which would be 189 MB):

  s[b,u,o]    = sum_{c,i} x[b,i,c] * (coef[c,u] * W[c,u,o,i])     (matmul, K=(c,i))
  v           = squash(s)
  G[ci,uo]    = sum_b x[b,i,c] * v[b,u,o]                          (matmul, K=b)
  agr[c,u]    = (1/B) * sum_{o,i} W[c,u,o,i] * G[(c,i),(u,o)]      (mult + selector matmul)
  b_logits   += agr ; coef = softmax_u(b_logits)                   (tiny, c-local)

Only cross-core traffic: AllGather of the per-core partial s once per routing
iteration (4 total).  The c-sharded agreement/logits state is fully
core-local.  Iteration 1's uniform coef=0.1 is folded into a 0.1 pre-scale of
the x operand used by the s-matmul (cancelled later by the 10x
coefficient-expansion constant).

v3 design:
- all matmuls in bf16 (1 cyc/row, 1-pass LDWEIGHTS); bf16 inputs halve the
  HBM upload.  The two b-half s accumulation groups live in separate 2KB
  PSUM zero regions (start=True marks the whole region pending-zero, so
  interleaved groups must not share a bank).
- the AllGather wire is fp8 e3m4 with a 0.5 pre-scale (values ~N(0,1.7),
  e3m4 max 30 - safe).  The rescale is folded into the squash: with
  s' = s/2, v = s' * sqrt(16*n2')/(1 + 4*n2'), using the ACT sqrt scale
  operand and a fused tensor_scalar mul-add - zero extra ops.  The last
  round's wire is configurable (fp8 by default; fp16 fallback).
- the post-AllGather gather of the 8 rank partials is 2 strided DMAs on the
  sync+scalar HWDGE queues; 8-way tree-sum on DVE with fp16 intermediates.
- routing logits b live in PSUM: the selector matmul accumulates them
  across rounds directly (start only on the first write), so the logits
  update costs no extra DVE op; Exp reads PSUM on the ACT engine.
- the agreement -> softmax -> W_eff -> next-round s-matmul chain is
  pipelined in 3 chunks of 3 k-tiles.
- dummy matmuls into a dedicated scratch PSUM bank keep the PE HAM
  clock-gate at 2.4 GHz: a burst gated on the gathered tile covers the
  tree/squash window, and a small burst per chunk covers the chunk phase.
"""

import os
import sys

# Prefer the Mesh collective algorithm for the small AllGathers if the
# runtime sees this env (no-op under the axon-tunneled remote runtime).
os.environ.setdefault("NEURON_RT_DBG_RDH_CC", "0")

if "/opt/trn_rl_repo" not in sys.path:
    sys.path.insert(0, "/opt/trn_rl_repo")

import numpy as np

import concourse.bacc as bacc
import concourse.tile as tile
from concourse import mybir
from concourse.bass_utils import run_bass_kernel_spmd

F32 = mybir.dt.float32
F16 = mybir.dt.float16
BF16 = mybir.dt.bfloat16
F8 = mybir.dt.float8e3            # e3m4: 4 mantissa bits, |max| ~30
WIRE_LAST_FP16 = True            # fp16 wire on the final AllGather

B = 256          # batch
IU = 8           # in_unit (i)
C = 1152         # input capsules
U = 10           # output capsules
O = 16           # unit size
N_CORES = 8
CL = C // N_CORES          # 144 local capsules
CI = CL * IU               # 1152 local (c,i) rows
K = CI // 128              # 9 contraction tiles
UO = U * O                 # 160
NROUTE = 4
NCH = 3                    # k-tiles per agreement chunk (K/3)
N_WARM = 28                # post-AllGather PE keep-warm dummy matmuls
N_WARM_CHUNK = 6           # per-chunk PE keep-warm dummy matmuls


def _build_program():
    nc = bacc.Bacc(
        "TRN2",
        target_bir_lowering=False,
        debug=False,
        enable_asserts=False,
        num_devices=N_CORES,
    )

    xp_d = nc.dram_tensor("xp", [128, K * B], BF16, kind="ExternalInput").ap()
    xb_d = nc.dram_tensor("xb", [128, 2 * CI], BF16, kind="ExternalInput").ap()
    w1_d = nc.dram_tensor("w1", [128, K * UO], BF16, kind="ExternalInput").ap()
    sel_d = nc.dram_tensor("sel", [128, 16], BF16, kind="ExternalInput").ap()
    exp_d = nc.dram_tensor("exp16", [16, 128], BF16, kind="ExternalInput").ap()
    out_d = nc.dram_tensor("out", [B, U, O, 1], F32, kind="ExternalOutput").ap()

    with tile.TileContext(nc) as tc:
        with (
            tc.tile_pool(name="persist", bufs=1) as pp,
            tc.tile_pool(name="work", bufs=2) as wp,
            tc.tile_pool(name="sps", bufs=1, space="PSUM") as sps,
            tc.tile_pool(name="gps", bufs=1, space="PSUM") as gps,
            tc.tile_pool(name="bps", bufs=1, space="PSUM") as bps,
            tc.tile_pool(name="cxps", bufs=1, space="PSUM") as cxps,
            tc.tile_pool(name="wps", bufs=1, space="PSUM") as wps,
            tc.tile_pool(name="dram", bufs=2, space="DRAM") as dram,
        ):
            # ---- persistent tiles ----
            xp_s = pp.tile([128, K * B], BF16, tag="xp")
            xb_s = pp.tile([128, 2 * CI], BF16, tag="xb")
            w1_s = pp.tile([128, K * UO], BF16, tag="w1")
            weff_s = pp.tile([128, K * UO], BF16, tag="weff")
            v_s = pp.tile([128, 2 * UO], BF16, tag="v")
            v_last = pp.tile([128, 2 * UO], F32, tag="vlast")
            pm2_s = pp.tile([128, K * U], BF16, tag="pm2")
            sel_s = pp.tile([128, 16], BF16, tag="sel")
            exp_s = pp.tile([16, 128], BF16, tag="exp16")
            scr = pp.tile([128, 2], F32, tag="scr")   # ACT table prewarm scratch

            # routing logits accumulate in PSUM via the selector matmul
            b_ps = bps.tile([16, K * U], F32, tag="b_ps")
            warm_ps = wps.tile([128, UO], F32, tag="warm")

            nc.gpsimd.memset(scr[:], 1.0)

            # ---- input loads (chunked so the round-0 s-matmul starts early) ----
            for j in range(3):
                kb3, ku3 = 3 * B, 3 * UO
                nc.sync.dma_start(
                    xp_s[:, j * kb3:(j + 1) * kb3], xp_d[:, j * kb3:(j + 1) * kb3]
                )
                nc.scalar.dma_start(
                    w1_s[:, j * ku3:(j + 1) * ku3], w1_d[:, j * ku3:(j + 1) * ku3]
                )
            nc.scalar.dma_start(sel_s[:], sel_d)
            nc.scalar.dma_start(exp_s[:], exp_d)
            nc.scalar.dma_start(xb_s[:, :CI], xb_d[:, :CI])
            nc.scalar.dma_start(xb_s[:, CI:], xb_d[:, CI:])

            def s_mm(s_ps, weff, ks):
                # the two b-half accumulation groups sit at offsets 0 and 512
                # f32 (separate 2KB zero regions) so they may interleave.
                for g in range(2):
                    for k in ks:
                        nc.tensor.matmul(
                            s_ps[:, g * 512: g * 512 + UO],
                            lhsT=xp_s[:, k * B + g * 128: k * B + (g + 1) * 128],
                            rhs=weff[:, k * UO:(k + 1) * UO],
                            start=(k == 0), stop=(k == K - 1),
                        )

            def stage_and_ag(s_ps, wire_dt):
                tag = "8" if wire_dt == F8 else "16"
                sw = wp.tile([128, 2 * UO], wire_dt, tag="sw" + tag)
                if wire_dt == F8:
                    # 0.5 pre-scale keeps the fp8 e3m4 wire well inside range;
                    # the squash rescales exactly.
                    nc.vector.tensor_scalar_mul(
                        sw[:].rearrange("p (g f) -> p g f", g=2),
                        s_ps[:].rearrange("p (g w) -> p g w", g=2)[:, :, :UO],
                        0.5,
                    )
                else:
                    nc.vector.tensor_copy(
                        sw[:].rearrange("p (g f) -> p g f", g=2),
                        s_ps[:].rearrange("p (g w) -> p g w", g=2)[:, :, :UO],
                    )
                cc_in = dram.tile([B, UO], wire_dt, tag="cc_in" + tag)
                cc_out = dram.tile(
                    [N_CORES * B, UO], wire_dt, tag="cc_out" + tag,
                    addr_space="Shared",
                )
                nc.sync.dma_start(
                    cc_in.opt().rearrange("(g p) f -> p g f", g=2), sw[:]
                )
                nc.gpsimd.collective_compute(
                    "AllGather",
                    mybir.AluOpType.bypass,
                    replica_groups=[list(range(N_CORES))],
                    ins=[cc_in.opt()],
                    outs=[cc_out.opt()],
                )
                return cc_out

            def wire_for(rnd_of_ag):
                last_ag = rnd_of_ag == NROUTE - 1
                return F16 if (WIRE_LAST_FP16 and last_ag) else F8

            # ---- round 0: s = (0.1 x)^T W, AllGather ----
            s_ps = sps.tile([128, 2 * 512], F32, tag="s_ps")
            s_mm(s_ps, w1_s, range(K))
            cc_out = stage_and_ag(s_ps, wire_for(0))

            for rnd in range(1, NROUTE + 1):
                last = rnd == NROUTE
                wire_dt = wire_for(rnd - 1)

                # ---- gather the 8 rank partials: 2 strided DMAs, tree-sum ----
                sg = wp.tile([128, 16 * UO], wire_dt, tag="sg" + ("8" if wire_dt == F8 else "16"))
                sgv = sg[:].rearrange("p (q g f) -> p q g f", q=8, g=2)
                ccv = cc_out.opt().rearrange("(q g p) f -> p q g f", q=8, g=2)
                nc.sync.dma_start(sgv[:, :4], ccv[:, :4])
                nc.scalar.dma_start(sgv[:, 4:], ccv[:, 4:])
                if not last:
                    # PE keep-warm: dummies gated on the gathered tile run
                    # during the tree/squash window and ramp the HAM clock
                    # before the G/s-matmul burst.
                    for _ in range(N_WARM):
                        nc.tensor.matmul(
                            warm_ps[:, :40],
                            lhsT=xp_s[:, :128],
                            rhs=sg[:, :80].bitcast(BF16),
                            start=True, stop=True,
                        )
                t1 = wp.tile([128, 8 * UO], F16, tag="t1")
                nc.vector.tensor_add(t1[:], sg[:, :8 * UO], sg[:, 8 * UO:])
                t2 = wp.tile([128, 4 * UO], F16, tag="t2")
                nc.vector.tensor_add(t2[:], t1[:, :4 * UO], t1[:, 4 * UO:])
                s_sb = wp.tile([128, 2 * UO], F16, tag="s_sb")
                nc.vector.tensor_add(s_sb[:], t2[:, :2 * UO], t2[:, 2 * UO:])

                # ---- squash (s' = s/2 on the fp8 wire):
                #      v = s' * sqrt(16 n2') / (1 + 4 n2') ----
                half = wire_dt == F8
                sq = wp.tile([128, 2 * UO], F32, tag="sq")
                nc.vector.tensor_mul(sq[:], s_sb[:], s_sb[:])
                n2 = wp.tile([128, 2 * U], F32, tag="n2")
                nc.vector.reduce_sum(
                    n2[:], sq[:].rearrange("p (t u o) -> p (t u) o", t=2, u=U),
                    axis=mybir.AxisListType.X,
                )
                rt = wp.tile([128, 2 * U], F32, tag="rt")
                nc.scalar.activation(
                    rt[:], n2[:], mybir.ActivationFunctionType.Sqrt,
                    scale=16.0 if half else 1.0,
                )
                if not last:
                    # prewarm the Exp ACT table while G runs (dep on rt orders it)
                    nc.scalar.activation(
                        scr[:, 1:2], rt[:, 0:1], mybir.ActivationFunctionType.Exp
                    )
                dn = wp.tile([128, 2 * U], F32, tag="dn")
                if half:
                    nc.gpsimd.tensor_scalar(
                        dn[:], n2[:], 4.0, 1.0,
                        mybir.AluOpType.mult, mybir.AluOpType.add,
                    )
                else:
                    nc.gpsimd.tensor_scalar_add(dn[:], n2[:], 1.0)
                rd = wp.tile([128, 2 * U], F32, tag="rd")
                nc.vector.reciprocal(rd[:], dn[:])
                f = wp.tile([128, 2 * U], F32, tag="f")
                nc.vector.tensor_mul(f[:], rt[:], rd[:])
                v_dst = v_last if last else v_s
                nc.vector.tensor_mul(
                    v_dst[:].rearrange("p (t u o) -> p t u o", t=2, u=U),
                    s_sb[:].rearrange("p (t u o) -> p t u o", t=2, u=U),
                    f[:].rearrange("p (t u) -> p t u", t=2).unsqueeze(3).broadcast_to((128, 2, U, O)),
                )

                if last:
                    nc.sync.dma_start(
                        out_d.rearrange("(g p) u o one -> p g (u o one)", g=2),
                        v_last[:].rearrange("p (t f) -> p t f", t=2),
                    )
                    break

                # ---- G[(c,i),(u,o)] = sum_b x*v, in 3 chunks of 3 k-tiles ----
                g_chunks = []
                for jc in range(3):
                    g_ps = gps.tile([128, NCH * UO], F32, tag=f"g_ps{jc}")
                    for mm in range(NCH):
                        m = NCH * jc + mm
                        for t in range(2):
                            nc.tensor.matmul(
                                g_ps[:, mm * UO:(mm + 1) * UO],
                                lhsT=xb_s[:, t * CI + m * 128: t * CI + (m + 1) * 128],
                                rhs=v_s[:, t * UO:(t + 1) * UO],
                                start=(t == 0), stop=(t == 1),
                            )
                    g_chunks.append(g_ps)

                # next round's s accumulator
                s_ps = sps.tile([128, 2 * 512], F32, tag="s_ps")

                # ---- per-chunk: agreement -> logits -> softmax -> W_eff -> s-matmul ----
                for j in range(3):
                    su = slice(j * NCH * U, (j + 1) * NCH * U)      # [3U] logits cols
                    suo = slice(j * NCH * UO, (j + 1) * NCH * UO)   # [3UO] weight cols

                    pm = wp.tile([128, NCH * UO], F32, tag=f"pm{j}")
                    nc.vector.tensor_mul(pm[:], g_chunks[j][:], w1_s[:, suo])
                    with nc.allow_low_precision(reason="routing logits tolerate bf16"):
                        nc.vector.reduce_sum(
                            pm2_s[:, su].rearrange("p (m u) -> p m u", m=NCH),
                            pm[:].rearrange("p (m u o) -> p m u o", m=NCH, u=U),
                            axis=mybir.AxisListType.X,
                        )
                    # logits accumulate in PSUM across rounds: start only on the
                    # very first write (marks the whole zero region pending, so
                    # round-1 chunks 1/2 overwrite-on-first-write), stop on the
                    # last agreement round's final chunk.
                    nc.tensor.matmul(
                        b_ps[:, su], lhsT=sel_s[:], rhs=pm2_s[:, su],
                        start=(rnd == 1 and j == 0),
                        stop=(rnd == NROUTE - 1 and j == 2),
                    )
                    eb = wp.tile([16, NCH * U], F32, tag=f"eb{j}")
                    nc.scalar.activation(
                        eb[:], b_ps[:, su], mybir.ActivationFunctionType.Exp
                    )
                    if j == 2:
                        # prewarm the Sqrt ACT table for the next squash
                        nc.scalar.activation(
                            scr[:16, 0:1], eb[:, 0:1], mybir.ActivationFunctionType.Sqrt
                        )
                    den = wp.tile([16, NCH], F32, tag=f"den{j}")
                    nc.vector.reduce_sum(
                        den[:], eb[:].rearrange("p (m u) -> p m u", m=NCH),
                        axis=mybir.AxisListType.X,
                    )
                    rden = wp.tile([16, NCH], F32, tag=f"rden{j}")
                    nc.vector.reciprocal(rden[:], den[:])
                    cn = wp.tile([16, NCH * U], BF16, tag=f"cn{j}")
                    nc.vector.tensor_mul(
                        cn[:].rearrange("p (m u) -> p m u", m=NCH),
                        eb[:].rearrange("p (m u) -> p m u", m=NCH),
                        rden[:].unsqueeze(2).broadcast_to((16, NCH, U)),
                    )
                    cx_ps = cxps.tile([128, NCH * U], F32, tag="cx")
                    nc.tensor.matmul(
                        cx_ps[:], lhsT=exp_s[:], rhs=cn[:], start=True, stop=True
                    )
                    nc.vector.tensor_mul(
                        weff_s[:, suo].rearrange("p (m u o) -> p m u o", m=NCH, u=U),
                        w1_s[:, suo].rearrange("p (m u o) -> p m u o", m=NCH, u=U),
                        cx_ps[:].rearrange("p (m u) -> p m u", m=NCH).unsqueeze(3).broadcast_to((128, NCH, U, O)),
                    )
                    s_mm(s_ps, weff_s, range(j * NCH, (j + 1) * NCH))
                    # keep the PE warm through the chunk phase: these fire
                    # after this chunk's s-matmul and fill the idle window
                    # until the next chunk's selector matmul is ready.
                    for _ in range(N_WARM_CHUNK):
                        nc.tensor.matmul(
                            warm_ps[:],
                            lhsT=weff_s[:, j * NCH * UO: j * NCH * UO + 128],
                            rhs=weff_s[:, j * NCH * UO: j * NCH * UO + UO],
                            start=True, stop=True,
                        )

                cc_out = stage_and_ag(s_ps, wire_for(rnd))

    nc.compile()
    return nc


_PROGRAM_CACHE = {}


def _get_program():
    if "nc" not in _PROGRAM_CACHE:
        _PROGRAM_CACHE["nc"] = _build_program()
    return _PROGRAM_CACHE["nc"]


def _make_in_maps(x, W):
    BF16_NP = mybir.dt.np(BF16)
    x = np.ascontiguousarray(x, dtype=np.float32)
    W = np.ascontiguousarray(W, dtype=np.float32)
    sel = np.zeros((128, 16), dtype=np.float32)
    for p in range(128):
        sel[p, p // IU] = 1.0 / B
    exp16 = np.zeros((16, 128), dtype=np.float32)
    for p in range(128):
        exp16[p // IU, p] = 10.0  # cancels the 0.1 pre-scale of xp
    sel = sel.astype(BF16_NP)
    exp16 = exp16.astype(BF16_NP)

    in_maps = []
    for core in range(N_CORES):
        c0 = core * CL
        xc = x[:, :, c0:c0 + CL]                    # [B, I, CL]
        Wc = W[c0:c0 + CL]                          # [CL, U, O, I]
        # xp[p, k*B + b] = 0.1 * x[b, i, c], ci = k*128+p = c_rel*8+i
        xp = 0.1 * xc.transpose(2, 1, 0).reshape(CI, B)
        xp = np.ascontiguousarray(
            xp.reshape(K, 128, B).transpose(1, 0, 2).reshape(128, K * B)
        ).astype(BF16_NP)
        # xb[p, t*CI + ci] = x[t*128+p, i, c]
        xb = xc.transpose(0, 2, 1).reshape(B, CI)
        xb = np.ascontiguousarray(
            xb.reshape(2, 128, CI).transpose(1, 0, 2).reshape(128, 2 * CI)
        ).astype(BF16_NP)
        # w1[p, k*UO + uo] = W[c, u, o, i]
        w1 = Wc.transpose(0, 3, 1, 2).reshape(CI, UO).reshape(K, 128, UO)
        w1 = np.ascontiguousarray(
            w1.transpose(1, 0, 2).reshape(128, K * UO)
        ).astype(BF16_NP)
        in_maps.append(
            {"xp": xp, "xb": xb, "w1": w1, "sel": sel, "exp16": exp16}
        )
    return in_maps


def kernel(x, W, _trace=False, _trace_kwargs=None):
    nc = _get_program()
    in_maps = _make_in_maps(x, W)
    res = run_bass_kernel_spmd(
        nc, in_maps, core_ids=list(range(N_CORES)), trace=_trace,
        **(_trace_kwargs or {}),
    )
    out = res.results[0]["out"].astype(np.float32).reshape(B, U, O, 1)
    if _trace:
        kernel.last_results = res
    return out
